# revision 1
# baseline (speedup 1.0000x reference)
"""Causal multi-head attention (B=4, S=1024, D=1024, H=16, hd=64) on 8 TRN2 cores.

Sharding: head-parallel. Core c owns heads {2c, 2c+1} for all batches, i.e.
d-columns [128c, 128c+128) of q/k/v/out. Each core runs independent causal
attention for its 8 (batch, head) pairs; no collectives.

v2 design (vs v1's ACT-only exp at a 38us ACT floor): the exp work is split
across ACT and DVE, the causal mask is folded into the DVE op, and the PE
transposes + DVE PSUM->SBUF copies are replaced by xbar DMA transposes.

  - qT/kT [128d, 1024s] fp16 tiles come straight from DRAM via
    dma_start_transpose (xbar, ~896ns per [1024,128] tensor).  No identity
    matmuls, no PSUM staging, no DVE copies.
  - scoresT blocks are computed by PE into 9 PSUM groups of 1024 cols per
    batch (3-deep ring, slot<->engine affinity; spans never cross a 512-col
    PSUM bank, and every bank is written by a single head's matmuls -- HW
    rejects mixed lhsT partition bases within one bank).
  - exp: 6 groups on ACT (exact exp LUT, scale folded), 3 groups on DVE via
    a Schraudolph-style bit-trick: fp16 bits of exp(SCALE*x) are
    round(x*A + B) with A = SCALE*1024*log2(e), B = (15-sigma)*1024.  The
    DVE op is scalar_tensor_tensor: out_i16 = (x + B/A) * tri, where tri is
    a per-element constant = A off-diagonal / A*(qr>=kc) on the 6 packed
    diagonal 128-blocks per head -> masked entries become exact +0.0 and the
    int16 result is bitcast as the fp16 expT tile (verified on HW: DVE
    converts fp32->int16 with round-to-nearest; values stay in [6k,25k] so
    no wrap/saturate).  Scores are O(6) so fp32/fp16 exp cannot overflow.
  - the first two diagonal blocks per head (query rows 0..255, few-key
    softmax rows where the ~3% bit-exp error is worst) go in an ACT group
    and are masked by gpsimd affine_select (Pool is otherwise idle).
    Measured end-to-end rel err ~6e-3 vs the 2e-2 gate.
  - out[qr, hd] and the softmax denominator come from one PE accumulation:
    lhsT = expT block slices [kc, qr], rhs = v_aug [kc, 65] (v plus a ones
    column, prepared host-side).  reciprocal + normalize stay on DVE
    (gpsimd has no PSUM port).
  - PV/normalize for batch b interleave with batch b+1's exp groups; the
    out store (fp16, host upcasts) is dispatched one section later so the
    in-order SP queue never stalls input loads on a late normalize; v_aug
    for all batches loads once at start; qkT xbar-transposes are issued a
    full section ahead (4-deep ring).  Per-iter engine busy: ACT ~24.9us,
    PE ~24us, DVE ~22.6us, DMA ~16us, Pool ~3us; TimelineSim single-shot
    38.7us / steady 25.1us vs the v1 baseline's 53.4/38.6 (HW-measured
    steady on a quiet machine: 24.8us/iter).

TRN2 instructions have one HW semaphore-wait slot; split_multi_waits()
legalizes multi-producer waits.
"""

import sys

sys.path.insert(0, "/opt/trn_rl_repo")

import numpy as np

import concourse.bass as bass
import concourse.mybir as mybir
import concourse.tile as tile
from concourse import bass_utils

B, S, D, H = 4, 1024, 1024, 16
HD = 64
NCORES = 8
HPC = H // NCORES          # heads per core = 2
CW = HPC * HD              # per-core d-column width = 128
P = 128                    # partitions
NT = S // P                # 8 s-tiles of 128
GCOLS = 1024               # psum exp-group width (2 banks)
NG = 9                     # exp groups per batch
SCALE = HD ** -0.5
FP32 = mybir.dt.float32
F16 = mybir.dt.float16
I16 = mybir.dt.int16
F16_NP = np.float16

# bit-exp constants: fp16 bits of exp(SCALE*x) ~= round(x*AEXP + BEXP)
SIGMA = 0.02
AEXP = float(SCALE * 1024.0 * np.log2(np.e))
BEXP = (15.0 - SIGMA) * 1024.0
BA = BEXP / AEXP

# off-diagonal span of score block (c, j): qr in [qs, qs+w), kc block j
_OFF = {(0, 0): (128, 384), (0, 1): (256, 256), (0, 2): (384, 128),
        (1, 0): (512, 512), (1, 1): (512, 512), (1, 2): (512, 512),
        (1, 3): (512, 512), (1, 4): (640, 384), (1, 5): (768, 256),
        (1, 6): (896, 128)}


def split_multi_waits(nc):
    """TRN2 TPB instructions carry exactly one semaphore wait slot; walrus
    refuses >1 on_wait per instruction.  Hoist extra waits onto standalone
    EventSemaphore instructions on the same engine, inserted right before the
    owning instruction (engines dispatch in order, so semantics are
    unchanged)."""
    ctr = [0]
    for fn in nc.m.functions:
        for blk in fn.blocks:
            insts = list(blk.instructions)
            out = []
            changed = False
            for inst in insts:
                si = inst.sync_info
                if si is not None and len(si.on_wait) > 1:
                    changed = True
                    waits = list(si.on_wait)
                    for w in waits[:-1]:
                        ev = mybir.InstEventSemaphore(
                            name=f"evw-split-{ctr[0]}", ins=[], outs=[]
                        )
                        ctr[0] += 1
                        ev.engine = inst.engine
                        ev.sync_info = mybir.SyncInfo(on_wait=[w], on_update=[])
                        out.append(ev)
                    inst.sync_info = mybir.SyncInfo(
                        on_wait=[waits[-1]], on_update=list(si.on_update)
                    )
                out.append(inst)
            if changed:
                for i, inst in enumerate(out):
                    existing = blk.instructions
                    if i < len(existing) and existing[i].name == inst.name:
                        continue
                    blk.instructions.insert(i, inst)


def _schedule():
    """Per-batch exp-group schedule (identical for all batches).

    Returns (groups, xd, xo):
      groups: emission-ordered list of dicts
        eng:  'dve' | 'act'
        spans: [(hl, c, j, qs, w, goff)]  QK matmul spans (psum offset goff)
        sel:  [goff] diag spans needing gpsimd affine_select (ACT groups)
        xbase: expT column base
      xd[(hl, qt)] -> expT col of the 128-wide diagonal span of query tile qt
      xo[(hl, c, j)] -> (expT col, qs) of the off-diagonal span of block (c,j)

    DVE groups pack the 6 bit-exp diagonal blocks of one head at goff
    0..768 (matching the tri constant layout) plus 768 cols of off-diag
    fill; ACT group 1 carries the 4 exact diagonal blocks (qt 0,1 both
    heads).  Spans never cross a 512-col PSUM bank.
    """
    def off(hl, c, j, goff):
        qs, w = _OFF[(c, j)]
        return (hl, c, j, qs, w, goff)

    def diag(hl, qt, goff):
        return (hl, qt // 4, qt, 128 * qt, 128, goff)

    # DVE groups: per head the 6 bit-exp diagonal blocks at goff 0..768
    # (matching tri's layout) + a 256-col off-diag fill; plus one pure
    # off-diag group.
    g_dve = []
    for hl in (0, 1):
        spans = [diag(hl, qt, 128 * (qt - 2)) for qt in range(2, 8)]
        spans.append(off(hl, 0, 1, 768))     # 256
        g_dve.append(dict(eng="dve_tri", spans=spans, sel=[]))
    g_dve.append(dict(eng="dve", sel=[], spans=[
        off(0, 1, 3, 0), off(1, 1, 3, 512)]))
    # ACT groups: 6 x 1024, one head per group.  HW constraint (verified by
    # bisection): all matmuls writing one 512-col PSUM bank must use the
    # same lhsT partition base, i.e. banks are head-homogeneous.
    g_act = []
    for hl in (0, 1):
        g_act.append(dict(eng="act", sel=[0, 128], spans=[
            diag(hl, 0, 0), diag(hl, 1, 128), off(hl, 1, 5, 256),
            off(hl, 1, 0, 512)]))
        g_act.append(dict(eng="act", sel=[], spans=[
            off(hl, 1, 1, 0), off(hl, 1, 2, 512)]))
        g_act.append(dict(eng="act", sel=[], spans=[
            off(hl, 0, 0, 0), off(hl, 0, 2, 384), off(hl, 1, 4, 512),
            off(hl, 1, 6, 896)]))
    ga_h0 = g_act[0:3]
    ga_h1 = g_act[3:6]
    # Emission order gives each psum ring slot (3-deep) a single consumer
    # engine: slots 0/2 -> ACT, slot 1 -> DVE, so every engine's next group
    # is already score-filled when it retires the previous one.
    ordered = [ga_h0[0], g_dve[0], ga_h1[0], ga_h0[1], g_dve[1], ga_h1[1],
               ga_h0[2], g_dve[2], ga_h1[2]]
    # drain order: the three groups not needed by the qr-lower-half PV
    # chunks (c1j1/j2 and c1j3 spans) go last, so the final batch's half-0
    # PV/normalize/store overlap them and only half-1 sits in the tail.
    # Slot<->engine affinity (0/2->ACT, 1->DVE) is preserved.
    final = [ga_h0[0], g_dve[0], ga_h0[2], ga_h1[0], g_dve[1], ga_h1[2],
             ga_h0[1], g_dve[2], ga_h1[1]]

    xd, xo = {}, {}
    for pos, g in enumerate(ordered):
        g["xbase"] = pos * GCOLS
        for (hl, c, j, qs, w, goff) in g["spans"]:
            if w == P and qs == 128 * j:
                xd[(hl, j)] = g["xbase"] + goff
            else:
                xo[(hl, c, j)] = (g["xbase"] + goff, qs)
    # sanity: every (qt, j) PV block resolves
    for hl in range(HPC):
        for qt in range(NT):
            assert (hl, qt) in xd
            for j in range(qt):
                assert (hl, 0 if qt < 4 else 1, j) in xo
    # alt schedule: one pure-off ACT group runs as DVE bit-exp instead.
    # Used on one batch per iteration so the ACT/DVE split balances at
    # iteration granularity (ACT 24.9 -> 23.9 us/iter, DVE 22.6 -> 23.8).
    alt = []
    for g in ordered:
        if g is ga_h1[1]:
            g2 = dict(g)
            g2["eng"] = "dve"
            alt.append(g2)
        else:
            alt.append(g)
    return ordered, alt, final, xd, xo


_GROUPS, _GROUPS_ALT, _GROUPS_FINAL, _XD, _XO = _schedule()


def build_program(repeat: int = 1, debug_stage: int = 4):
    # debug_stage: 1=loads only, 2=+exp (dump expT), 3=+pv via SP store, 4=full
    nc = bass.Bass(trn_type="TRN2")
    qk_d = nc.dram_tensor("qk", [B, 2, S, CW], F16, kind="ExternalInput")
    # value_aug is host-prepacked to the exact SBUF layout for ALL batches
    # ([P, B*HPC*NT*(HD+1)]) and loaded once at program start: per-batch
    # DMAs then reduce to qkT + out, which keeps the HWDGE lane-order waits
    # off the batch critical path.
    va_d = nc.dram_tensor("value_aug", [P, B * HPC * NT * (HD + 1)], F16,
                          kind="ExternalInput")
    # fp16 output (host upcasts): halves the store on the serial DMA chain.
    o_d = nc.dram_tensor("attn_out", [B, S, CW], F16, kind="ExternalOutput")
    if debug_stage < 3:
        dbg_d = nc.dram_tensor("dbg", [P, NG * GCOLS], F16, kind="ExternalOutput")

    with tile.TileContext(nc) as tc:
        with (
            tc.tile_pool(name="const", bufs=1) as constp,
            tc.tile_pool(name="trp", bufs=4) as trp,
            tc.tile_pool(name="expp", bufs=2) as expp,
            tc.tile_pool(name="outp", bufs=3) as outp,
            tc.tile_pool(name="smallp", bufs=4) as smallp,
            tc.tile_pool(name="psc", bufs=3, space="PSUM") as psc,
            tc.tile_pool(name="pout", bufs=2, space="PSUM") as pout,
        ):
            # tri is generated on-chip by the (idle) Pool engine during the
            # DMA fill: memset to A, then zero the 6 causal triangles.
            tri = constp.tile([P, GCOLS], F16)
            nc.gpsimd.memset(tri[:], AEXP)
            for t in range(6):
                nc.gpsimd.affine_select(
                    out=tri[:, P * t:P * (t + 1)],
                    in_=tri[:, P * t:P * (t + 1)],
                    compare_op=mybir.AluOpType.is_ge,
                    fill=0.0, base=0, pattern=[[1, P]],
                    channel_multiplier=-1,
                )
            va_sb = constp.tile([P, B * HPC * NT * (HD + 1)], F16)
            # Dummy 1-col exp issued first: the ~2.7us exp table-set load
            # happens during the DMA fill instead of before the first real
            # exp.  Reads whatever is in the (unloaded) tri tile; result
            # discarded into scratch.
            warm = constp.tile([P, 1], FP32)
            nc.scalar.activation(warm[:], tri[:, 0:1],
                                 mybir.ActivationFunctionType.Exp)
            # PE p-state pre-ramp: ~3.8us of dummy matmuls (at the cold
            # 0.65/1.2GHz clocks) complete inside the ~5us DMA fill, so the
            # first real QK runs at the full 2.4GHz instead of restarting
            # the 3us ramp.
            warm_mm = constp.tile([P, 512], F16)
            nc.vector.memset(warm_mm[:], 0.0)
            warm_ps = psc.tile([P, GCOLS], FP32, tag="ps", name="warm_ps")
            for _ in range(8):
                nc.tensor.matmul(warm_ps[:, 0:512], warm_mm[0:64, 0:128],
                                 warm_mm[0:64, 0:512], start=True, stop=True)

            def emit_pv_chunk(ctx, out_sb, hl, c):
                b, expT = ctx
                v_view = va_sb.rearrange(
                    "p (v h j e) -> p v h j e", v=B, h=HPC, e=HD + 1)[:, b, hl]
                po4 = pout.tile([P, 4 * (HD + 1)], FP32, tag="po")
                for qi in range(4):
                    qt = 4 * c + qi
                    for j in range(qt + 1):
                        if j == qt:
                            o = _XD[(hl, qt)]
                        else:
                            x0, sqs = _XO[(hl, 0 if qt < 4 else 1, j)]
                            o = x0 + P * qt - sqs
                        nc.tensor.matmul(
                            po4[:, qi * (HD + 1):(qi + 1) * (HD + 1)],
                            expT[:, o:o + P],
                            v_view[:, j, :],
                            start=(j == 0), stop=(j == qt),
                        )
                po_v = po4.rearrange("p (t e) -> p t e", e=HD + 1)
                recip4 = smallp.tile([P, 4], FP32, tag="recip")
                rv = recip4.rearrange("p (t o) -> p t o", o=1)
                nc.vector.reciprocal(rv, po_v[:, :, HD:HD + 1])
                out_v = out_sb.rearrange("p (t j) -> p t j", j=P)[
                    :, c * 4:(c + 1) * 4, hl * HD:(hl + 1) * HD]
                nc.vector.tensor_mul(
                    out_v, po_v[:, :, 0:HD], rv.broadcast_to((P, 4, HD)))

            def emit_pv(ctx):
                b, expT = ctx
                out_sb = outp.tile([P, S], F16, tag="out_sb")
                for hl, c in ((0, 0), (0, 1), (1, 0), (1, 1)):
                    emit_pv_chunk(ctx, out_sb, hl, c)
                return (b, out_sb)

            def emit_out(ctx):
                # dispatched one section later, after the next batch's input
                # loads: by then the normalize is done, so the in-order SP
                # queue never stalls loads behind an out store (and the Pool
                # queue stays free for the selects, which gate PV).
                b, out_sb = ctx
                nc.sync.dma_start(
                    o_d[b].rearrange("(t p) j -> p t j", p=P),
                    out_sb.rearrange("p (t j) -> p t j", j=CW),
                )

            def load_qkT(b):
                # one xbar transpose loads q and k: [2*S, CW] -> [CW, 2*S]
                qkT = trp.tile([P, 2 * S], F16, tag="qkT", name="qkT")
                nc.sync.dma_start_transpose(
                    qkT, qk_d[b].rearrange("x s j -> (x s) j"))
                return qkT

            prev = None
            prev_out = None
            pending_qkT = load_qkT(0)
            for b_rep in range(repeat * B):
                b = b_rep % B
                qkT = pending_qkT
                qT = qkT[:, 0:S]
                kT = qkT[:, S:2 * S]
                # qkT for the NEXT batch is issued a full section early (ring
                # of 4): its transfer and the 900ns DMA-completion semaphore
                # propagation fully overlap the previous batch's compute.
                if b_rep + 1 < repeat * B:
                    pending_qkT = load_qkT((b_rep + 1) % B)
                if b_rep == 0:
                    nc.sync.dma_start(va_sb, va_d[:])
                if prev_out is not None:
                    emit_out(prev_out)
                    prev_out = None
                expT = expp.tile([P, NG * GCOLS], F16, tag="expT")
                dbg_ops = globals().get("_DBG_OPS", ("act", "dve", "sel"))
                # PV/normalize chunks of the previous batch interleave with
                # this batch's exp groups so the norms (and thus the out
                # store's SP dispatch) complete mid-section instead of at
                # the end, keeping the next qkT load unblocked.
                do_pv = debug_stage >= 3 and prev is not None
                pv_out_sb = None
                if do_pv:
                    pv_out_sb = outp.tile([P, S], F16, tag="out_sb",
                                          name="out_sb")
                pv_chunks = [(0, 0), (0, 1), (1, 0), (1, 1)]
                for gi, g in enumerate(_GROUPS):
                    xb = g["xbase"]
                    ps = psc.tile([P, GCOLS], FP32, tag="ps")
                    for (hl, c, j, qs, w, goff) in g["spans"]:
                        hp = hl * HD
                        nc.tensor.matmul(
                            ps[:, goff:goff + w],
                            kT[hp:hp + HD, j * P:(j + 1) * P],
                            qT[hp:hp + HD, qs:qs + w],
                            start=True, stop=True,
                        )
                    if g["eng"] == "dve_tri":
                        if "dve" not in dbg_ops:
                            continue
                        nc.vector.scalar_tensor_tensor(
                            expT[:, xb:xb + GCOLS].bitcast(I16),
                            ps[:], BA, tri[:],
                            mybir.AluOpType.add, mybir.AluOpType.mult,
                        )
                    elif g["eng"] == "dve":
                        if "dve" not in dbg_ops:
                            continue
                        nc.vector.tensor_scalar(
                            expT[:, xb:xb + GCOLS].bitcast(I16),
                            ps[:], AEXP, BEXP,
                            mybir.AluOpType.mult, mybir.AluOpType.add,
                        )
                    else:
                        if "act" not in dbg_ops:
                            continue
                        ds = g.get("dsplit", GCOLS)
                        nc.scalar.activation(
                            expT[:, xb:xb + ds], ps[:, 0:ds],
                            mybir.ActivationFunctionType.Exp, scale=SCALE,
                        )
                        if ds < GCOLS:
                            # balance: tail of this group exps on DVE
                            nc.vector.tensor_scalar(
                                expT[:, xb + ds:xb + GCOLS].bitcast(I16),
                                ps[:, ds:GCOLS], AEXP, BEXP,
                                mybir.AluOpType.mult, mybir.AluOpType.add,
                            )
                        if "sel" not in dbg_ops:
                            continue
                        for so in g["sel"]:
                            nc.gpsimd.affine_select(
                                out=expT[:, xb + so:xb + so + P],
                                in_=expT[:, xb + so:xb + so + P],
                                compare_op=mybir.AluOpType.is_ge,
                                fill=0.0, base=0, pattern=[[1, P]],
                                channel_multiplier=-1,
                            )
                    if do_pv and gi in (2, 4, 6, 8):
                        hl, c = pv_chunks[gi // 2 - 1]
                        emit_pv_chunk(prev, pv_out_sb, hl, c)
                if do_pv:
                    prev_out = (prev[0], pv_out_sb)
                prev = (b, expT)
                if debug_stage < 3:
                    break
            if debug_stage <= 2:
                if debug_stage == 2:
                    nc.sync.dma_start(dbg_d[:], expT[:])
                else:
                    nc.sync.dma_start(dbg_d[:, 0:2 * S], qkT[:])
            else:
                if prev_out is not None:
                    emit_out(prev_out)
                # drain: final batch's PV in qr-halves, each half's store
                # overlapping the other half's compute.
                b_l = prev[0]
                fin_sb = outp.tile([P, S], F16, tag="out_sb", name="out_sb")
                ov = o_d[b_l].rearrange("(t p) j -> p t j", p=P)
                sv = fin_sb.rearrange("p (t j) -> p t j", j=CW)
                for half in range(2):
                    for hl in range(HPC):
                        emit_pv_chunk(prev, fin_sb, hl, half)
                    nc.sync.dma_start(ov[:, half * 4:(half + 1) * 4],
                                      sv[:, half * 4:(half + 1) * 4])
    split_multi_waits(nc)
    return nc


def _tri_const():
    idx = np.arange(P)
    blk = (idx[None, :] >= idx[:, None]).astype(np.float16) * np.float16(AEXP)
    tri = np.empty((P, GCOLS), np.float16)
    tri[:, :6 * P] = np.tile(blk, (1, 6))
    tri[:, 6 * P:] = np.float16(AEXP)
    return tri


_TRI = _tri_const()


def make_in_maps(query, key, value):
    query = np.asarray(query, dtype=np.float32)
    key = np.asarray(key, dtype=np.float32)
    value = np.asarray(value, dtype=np.float32)
    in_maps = []
    for c in range(NCORES):
        sl = slice(c * CW, (c + 1) * CW)
        # packed v_aug: [P, ((b*2 + hl)*8 + j)*65 + e] = v[b, 128j+p, 64hl+e],
        # ones at e=64
        v_shard = value[:, :, sl].reshape(B, NT, P, HPC, HD)
        v_aug = np.ones((P, B, HPC, NT, HD + 1), dtype=F16_NP)
        v_aug[..., :HD] = v_shard.transpose(2, 0, 3, 1, 4).astype(F16_NP)
        qk = np.stack([query[:, :, sl], key[:, :, sl]], axis=1).astype(F16_NP)
        in_maps.append(
            {
                "qk": np.ascontiguousarray(qk),
                "value_aug": v_aug.reshape(P, B * HPC * NT * (HD + 1)),
            }
        )
    return in_maps


_RUNNER = None


def _get_runner():
    """Build the Bass program once and return a cached jitted 8-core runner
    (mirrors bass2jax.run_bass_via_pjrt's shard_map path; re-invoking
    run_bass_kernel_spmd would re-trace and re-jit on every call)."""
    global _RUNNER
    if _RUNNER is not None:
        return _RUNNER
    import jax
    from jax.sharding import Mesh, PartitionSpec
    from jax.experimental.shard_map import shard_map
    from concourse import bass2jax

    nc = build_program()
    bass2jax.install_neuronx_cc_hook()

    partition_name = nc.partition_id_tensor.name if nc.partition_id_tensor else None
    in_names, out_names, out_avals, zero_outs = [], [], [], []
    for alloc in nc.m.functions[0].allocations:
        if not isinstance(alloc, mybir.MemoryLocationSet):
            continue
        name = alloc.memorylocations[0].name
        if alloc.kind == "ExternalInput":
            if name != partition_name:
                in_names.append(name)
        elif alloc.kind == "ExternalOutput":
            shape = tuple(alloc.tensor_shape)
            dtype = mybir.dt.np(alloc.dtype)
            out_names.append(name)
            out_avals.append(jax.core.ShapedArray(shape, dtype))
            zero_outs.append(np.zeros(shape, dtype))
    n_params = len(in_names)
    all_in_names = list(in_names) + list(out_names)
    if partition_name is not None:
        all_in_names.append(partition_name)

    def _body(*args):
        operands = list(args)
        if partition_name is not None:
            operands.append(bass2jax.partition_id_tensor())
        outs = bass2jax._bass_exec_p.bind(
            *operands,
            out_avals=tuple(out_avals),
            in_names=tuple(all_in_names),
            out_names=tuple(out_names),
            lowering_input_output_aliases=(),
            sim_require_finite=True,
            sim_require_nnan=True,
            nc=nc,
        )
        return tuple(outs)

    devices = jax.devices()[:NCORES]
    mesh = Mesh(np.asarray(devices), ("core",))
    spec = PartitionSpec("core")
    fn = jax.jit(
        shard_map(_body, mesh=mesh,
                  in_specs=(spec,) * (n_params + len(out_names)),
                  out_specs=(spec,) * len(out_names), check_rep=False),
        keep_unused=True,
    )
    _RUNNER = (fn, in_names, out_names, out_avals, zero_outs)
    return _RUNNER


def _concat_inputs(query, key, value):
    """Vectorized equivalent of concatenating make_in_maps() over cores:
    returns {name: [(8*dim0), ...] array} keyed like the ExternalInputs."""
    q16 = np.asarray(query, dtype=F16_NP).reshape(B, S, NCORES, CW)
    k16 = np.asarray(key, dtype=F16_NP).reshape(B, S, NCORES, CW)
    # qk: per core [B, 2, S, CW] -> concat [(8B), 2, S, CW]
    qk = np.stack(
        [q16.transpose(2, 0, 1, 3), k16.transpose(2, 0, 1, 3)], axis=2
    ).reshape(NCORES * B, 2, S, CW)
    v16 = np.asarray(value, dtype=F16_NP).reshape(B, NT, P, NCORES, HPC, HD)
    v_aug = np.ones((NCORES, P, B, HPC, NT, HD + 1), dtype=F16_NP)
    v_aug[..., :HD] = v16.transpose(3, 2, 0, 4, 1, 5)
    v_aug = v_aug.reshape(NCORES * P, B * HPC * NT * (HD + 1))
    return {
        "qk": np.ascontiguousarray(qk),
        "value_aug": np.ascontiguousarray(v_aug),
    }


def kernel(query: np.ndarray, key: np.ndarray, value: np.ndarray) -> np.ndarray:
    fn, in_names, out_names, out_avals, zero_outs = _get_runner()
    cat = _concat_inputs(query, key, value)
    concat_in = [cat[name] for name in in_names]
    concat_zeros = [
        np.zeros((NCORES * z.shape[0], *z.shape[1:]), z.dtype) for z in zero_outs
    ]
    out_arrs = fn(*concat_in, *concat_zeros)
    oi = out_names.index("attn_out")
    full = np.asarray(out_arrs[oi]).reshape(NCORES, *out_avals[oi].shape)
    return np.concatenate(list(full), axis=2).astype(np.float32)

